# revision 40
# baseline (speedup 1.0000x reference)
"""Self-contained TRN2 kernel for nn_Block_41695542510261 (dense transformer block).

Accepts FULL unsharded inputs, distributes across 8 NeuronCores internally
(2 cores per batch element, causal-balanced 64-row query chunks), returns
the FULL [4, 1024, 1024] output.
"""
import sys, os
for _p in ('/opt/trn_rl_repo', '/root/.axon_site/_ro/trn_rl_repo'):
    if os.path.isdir(_p) and _p not in sys.path:
        sys.path.insert(0, _p)
"""Transformer block kernel for TRN2 — 8-core SPMD, feature-major layout.

Reference: pre-LN attention block + SwiGLU FFN, B=4 T=1024 C=1024 H=16 D=64 DFF=4096.

Sharding: core c handles batch b=c//2, parity par=c%2. Each batch's 16
64-row query chunks split by parity; odd cores receive x with adjacent
64-column blocks swapped so "own" tokens sit at even block positions.
Key order inside each 128-key tile is permuted consistently for K/V/mask.

v2 design (vs the mp-major baseline):
- kt-major attention: for each head pair, scores loop over the 8 key
  tiles once with causally-trimmed moving width (512/512/384/384/...),
  both heads of the pair packed onto disjoint PE row groups. exp is one
  ScalarE call per key tile covering both heads ([128, 2, N]). AV
  accumulates over key tiles into a single [65, 512] PSUM bank per head
  via per-element has_written sub-region accumulation.
- Deferred softmax normalization: the denominator row (ones-column of V)
  is reciprocated straight out of PSUM with the fast custom-DVE
  reciprocal into a gathered r_all[16, 512]; hi/lo bf16 split is batched;
  normalization is applied just before Wo with K=2 selector-matmul
  broadcasts + one in-place DVE multiply per feature tile. This removes
  the per-group [1,256] DVE reciprocal/cast/sub chains that saturated
  the Vector engine and caused PE HAM clock oscillation.
- No keep_warm dummy matmuls; LN stats/apply stages are interleaved with
  V projections / Wo so the PE stream stays dense. LN2 stats ride inside
  the Wo loop. ACT table loads (sqrt/exp/sqrt/silu) are forced early via
  dummy activations so they hide under matmul phases.
"""
import contextlib
import json
import numpy as np
import ml_dtypes

import concourse.bass as bass
import concourse.mybir as mybir
import concourse.tile as tile

f32 = mybir.dt.float32
bf16 = mybir.dt.bfloat16
AF = mybir.ActivationFunctionType

C = 1024        # d_model
T = 1024        # seq len
H = 16          # heads
D = 64          # head dim
DFF = 4096
TOK = 512       # own tokens per core
NCT = C // 128  # 8 c tiles
NTT = T // 128  # 8 token (key) tiles
NP = H // 2     # 8 head pairs
NCH = 8         # q chunks per core (64 rows each)
EPS = 1e-5
SM_SCALE = 1.0 / 32.0  # 1/sqrt(d_model)


def split_multiwaits(bir_bytes: bytes) -> bytes:
    """Split multi-wait instructions into single-wait EventSemaphore
    carriers placed just before them on the same engine. This walrus
    build has one sync-wait slot for several ISA structs (self-loading
    matmuls, direct DMAs, drains)."""
    m = json.loads(bir_bytes)
    ctr = 0
    for f in m['functions']:
        for blk in f.get('blocks', []):
            insts = blk.get('instructions', [])
            out = []
            changed = False
            for i in insts:
                si = i.get('sync_info')
                w = (si or {}).get('on_wait') or []
                if len(w) > 1:
                    for extra in w[:-1]:
                        ctr += 1
                        out.append({
                            'debug': i.get('debug'),
                            'engine': i['engine'],
                            'ins': [], 'outs': [],
                            'name': f'I-esw-{ctr}',
                            'opcode': 'EventSemaphore',
                            'sync_info': {'on_update': [], 'on_wait': [extra]},
                        })
                    si['on_wait'] = [w[-1]]
                    changed = True
                out.append(i)
            if changed:
                blk['instructions'] = out
    return json.dumps(m).encode()


def patch_nc(nc):
    orig = nc.to_json_bytes
    nc.to_json_bytes = lambda: split_multiwaits(orig())
    return nc


def build_nc(causal=True, sz=None, silu_act=True):
    sz = sz or {}
    nc = bass.Bass(trn_type="TRN2", target_bir_lowering=False, debug=False)

    xt = nc.dram_tensor("xt", [C, T], f32, kind="ExternalInput")
    if causal:
        maskt = nc.dram_tensor("maskt", [128, 2, 2, 128], bf16, kind="ExternalInput")
    else:
        maskt = nc.dram_tensor("maskt", [NTT, 128, 2, TOK], bf16, kind="ExternalInput")
    wq = nc.dram_tensor("wq", [NP, C, 128], bf16, kind="ExternalInput")
    wk = nc.dram_tensor("wk", [NP, C, 128], bf16, kind="ExternalInput")
    wv = nc.dram_tensor("wv", [4, C, 256], bf16, kind="ExternalInput")
    wo = nc.dram_tensor("wo", [NCT, C, 128], bf16, kind="ExternalInput")
    w1 = nc.dram_tensor("w1", [32, C, 128], bf16, kind="ExternalInput")
    w2 = nc.dram_tensor("w2", [32, C, 128], bf16, kind="ExternalInput")
    w3 = nc.dram_tensor("w3", [NCT, DFF, 128], bf16, kind="ExternalInput")
    g1p = nc.dram_tensor("g1p", [128, NCT], f32, kind="ExternalInput")
    be1p = nc.dram_tensor("be1p", [128, NCT], f32, kind="ExternalInput")
    g2p = nc.dram_tensor("g2p", [128, NCT], f32, kind="ExternalInput")
    be2p = nc.dram_tensor("be2p", [128, NCT], f32, kind="ExternalInput")
    bop = nc.dram_tensor("bop", [128, NCT], f32, kind="ExternalInput")
    b3p = nc.dram_tensor("b3p", [128, NCT], f32, kind="ExternalInput")
    b1p = nc.dram_tensor("b1p", [128, 32], f32, kind="ExternalInput")
    b2p = nc.dram_tensor("b2p", [128, 32], f32, kind="ExternalInput")
    sel100p = nc.dram_tensor("sel100p", [100, NCT, 128], bf16, kind="ExternalInput")
    out = nc.dram_tensor("out", [C, TOK], f32, kind="ExternalOutput")

    def qoff(kt):
        return 128 * (kt // 2) if causal else 0

    with tile.TileContext(nc) as tc, contextlib.ExitStack() as ctx:
        consts = ctx.enter_context(tc.tile_pool(name="consts", bufs=1))
        perB = ctx.enter_context(tc.tile_pool(name="perB", bufs=1))
        w12p = ctx.enter_context(tc.tile_pool(name="w12", bufs=3))
        w3p = ctx.enter_context(tc.tile_pool(name="w3_sb", bufs=4))

        # ---- constants ----
        ones_row = consts.tile([1, 128], bf16)
        nc.vector.memset(ones_row, 1.0)
        ones_col = consts.tile([128, 1], bf16)
        nc.vector.memset(ones_col, 1.0)
        ones_colf = consts.tile([128, 1], f32)
        nc.vector.memset(ones_colf, 1.0)
        eps_t = consts.tile([1, 1], f32)
        nc.vector.memset(eps_t, EPS)
        # per-ci head-half selector for the normalization broadcast:
        # pnrm(ci)[p, q] = r[group 2ci, q] for p<64 else r[group 2ci+1, q].
        # r group g sits at partition 32*(g//4) + g%4 — quarters at 32-aligned
        # bases so the batched reciprocal/hi/lo DVE slices are legal.
        sel100 = consts.tile([100, NCT, 128], bf16)
        warm_t = consts.tile([128, 256], bf16)
        nc.vector.memset(warm_t, 0.0)
        # scratch for ACT table-preload dummies
        pre_in = consts.tile([1, 8], f32)
        nc.vector.memset(pre_in, 1.0)
        pre_out = consts.tile([1, 8], f32)

        def act_preload(func):
            nc.scalar.activation(pre_out, pre_in, func)

        def act_rsqrt(out_ap, in_ap, bias_ap):
            # rs = 1/sqrt(in + eps) in one ScalarE op. The bass helper
            # refuses Rsqrt; emit the instruction directly.
            se = nc.scalar
            ins = [se.lower_ap(in_ap), se.lower_ap(bias_ap),
                   mybir.ImmediateValue(dtype=f32, value=1.0),
                   mybir.ImmediateValue(dtype=f32, value=0.0)]
            return se.add_instruction(mybir.InstActivation(
                name=nc.get_next_instruction_name(),
                func=AF.Rsqrt, ins=ins, outs=[se.lower_ap(out_ap)]))

        def keep_warm(pool, n):
            # dependency-free matmuls that fill PE stalls so the HAM clock
            # gate stays at 2.4 GHz through DVE-paced stretches
            wp = pool.tile([128, 256], f32, tag="wp")
            for _ in range(n):
                nc.tensor.matmul(wp, lhsT=warm_t[:, 0:128], rhs=warm_t,
                                 start=True, stop=True)

        g1s = consts.tile([128, NCT], f32)
        nc.sync.dma_start(out=g1s, in_=g1p[:, :])
        be1s = consts.tile([128, NCT], f32)
        nc.sync.dma_start(out=be1s, in_=be1p[:, :])
        g2s = consts.tile([128, NCT], f32)
        nc.sync.dma_start(out=g2s, in_=g2p[:, :])
        be2s = consts.tile([128, NCT], f32)
        nc.sync.dma_start(out=be2s, in_=be2p[:, :])
        bos = consts.tile([128, NCT], f32)
        nc.sync.dma_start(out=bos, in_=bop[:, :])
        b3s = consts.tile([128, NCT], f32)
        nc.sync.dma_start(out=b3s, in_=b3p[:, :])
        b1s = consts.tile([128, 32], f32)
        nc.sync.dma_start(out=b1s, in_=b1p[:, :])
        b2s = consts.tile([128, 32], f32)
        nc.sync.dma_start(out=b2s, in_=b2p[:, :])
        if causal:
            mk = consts.tile([128, 2, 2, 128], bf16)
            nc.sync.dma_start(out=mk, in_=maskt[:, :, :, :])
        else:
            mk = consts.tile([128, NTT, 2, TOK], bf16)
            nc.sync.dma_start(
                out=mk, in_=maskt[:, :, :, :].rearrange("kt p x q -> p kt x q"))

        # ---- phase-B persistent tiles (live to the end) ----
        x2T = perB.tile([128, NCT, TOK], f32)
        h2T = perB.tile([128, NCT, TOK], bf16)
        outT = perB.tile([128, NCT, TOK], f32)

        # own-token columns (even 64-blocks) of [:, ct, :]
        def own(tl, ct):
            return tl[:, ct, :].rearrange(
                "p (j two i) -> p j two i", two=2, i=64)[:, :, 0, :]

        # ---------- feature-major layer norm stages ----------
        # stats: per ct emit bf16 cast (alternating ACT/DVE), square, and
        # the two ones-matmul accumulation chains. Returns psum stat tiles.
        def ln_stats_full(src, ntok, sqp, stps, tag):
            # one [33, ntok] PSUM tile: mean row at partition 0 (col-group 0),
            # ex2 row at partition 32 (col-group 32) — the two M=1 chains run
            # concurrently on disjoint PE column groups
            st = stps.tile([33, ntok], f32, tag="stat", name=f"stat{tag}")
            for ct in range(NCT):
                sq = sqp.tile([128, ntok], bf16, tag="sq")
                nc.vector.tensor_mul(sq, src(ct), src(ct))
                nc.tensor.matmul(st[0:1, :], lhsT=ones_colf, rhs=src(ct),
                                 tile_position=(0, 0),
                                 start=(ct == 0), stop=(ct == NCT - 1))
                nc.tensor.matmul(st[32:33, :], lhsT=ones_col, rhs=sq,
                                 tile_position=(0, 32),
                                 start=(ct == 0), stop=(ct == NCT - 1))
            return st

        # statmath: psum stats -> bf16 hi/lo pairs of mu and 1/sd
        def ln_statmath(stat_ps, ntok, stss):
            # stat_ps: [33, ntok] psum (row 0 = sum x, row 32 = sum x^2);
            # both stats land in base-0 tiles (2-input DVE ops require
            # equal base partitions)
            mu = stss.tile([1, ntok], f32, tag="mu")
            nc.scalar.mul(mu, stat_ps[0:1, :], 1.0 / C)
            var = stss.tile([1, ntok], f32, tag="var")
            nc.scalar.mul(var, stat_ps[32:33, :], 1.0 / C)
            musq = stss.tile([1, ntok], f32, tag="musq")
            nc.vector.tensor_mul(musq, mu, mu)
            nc.vector.tensor_sub(var, var, musq)
            rs = stss.tile([1, ntok], f32, tag="rs")
            act_rsqrt(rs, var, eps_t)
            mu_b = stss.tile([1, ntok], bf16, tag="mu_b")
            nc.vector.tensor_copy(out=mu_b, in_=mu)
            rs_b = stss.tile([1, ntok], bf16, tag="rs_b")
            nc.vector.tensor_copy(out=rs_b, in_=rs)
            return mu_b, rs_b

        # broadcast hi/lo stats across partitions via K=1 PE outer products
        def ln_bc(sm, ntok, bcp, tag):
            mu_b, rs_b = sm
            mu_bc = bcp.tile([128, ntok], f32, tag="mu_bc")
            rs_bc = bcp.tile([128, ntok], f32, tag="rs_bc")
            nc.tensor.matmul(mu_bc, lhsT=ones_row, rhs=mu_b,
                             start=True, stop=True)
            nc.tensor.matmul(rs_bc, lhsT=ones_row, rhs=rs_b,
                             start=True, stop=True)
            return mu_bc, rs_bc

        def ln_apply(src, dst, ntok, mu_bc, rs_bc, gs, bes, skip_affine, tmpp):
            for ct in range(NCT):
                tmp = tmpp.tile([128, ntok], f32, tag="lntmp")
                nc.vector.tensor_sub(tmp, src(ct), mu_bc)
                if skip_affine:
                    nc.vector.tensor_mul(dst(ct), tmp, rs_bc)
                else:
                    nc.vector.tensor_mul(tmp, tmp, rs_bc)
                    nc.scalar.activation(dst(ct), tmp, AF.Identity,
                                         bias=bes[:, ct:ct + 1],
                                         scale=gs[:, ct:ct + 1])

        with tc.tile_pool(name="perA", bufs=1) as perA:
            # ---- phase-A persistent tiles ----
            xT = perA.tile([128, NCT, T], f32)
            hT = perA.tile([128, NCT, T], bf16)
            ownhT = perA.tile([128, NCT, TOK], bf16)
            v_all = perA.tile([128, NTT, H, 65], bf16)
            ctxT = perA.tile([128, NCT, TOK], bf16)
            d_all = perA.tile([100, TOK], f32)
            r_all = perA.tile([100, TOK], f32)
            r_hi = perA.tile([100, TOK], bf16)
            # most partitions are never written but streamed by the K=4
            # selector matmuls — zero them so garbage/NaN cannot leak in
            nc.vector.memset(r_hi, 0.0)

            act_preload(AF.Sqrt)  # first table load during the x DMA

            for th in range(2):
                for ct in range(NCT):
                    nc.sync.dma_start(
                        out=xT[:, ct, th * 512:(th + 1) * 512],
                        in_=xt[ct * 128:(ct + 1) * 128, th * 512:(th + 1) * 512])

            # ---------- LN1 + V projections ----------
            with tc.tile_pool(name="ln_sq", bufs=2) as sqp, \
                 tc.tile_pool(name="ln_st", bufs=1, space="PSUM") as stps, \
                 tc.tile_pool(name="warm_ps", bufs=1, space="PSUM") as wmp, \
                 tc.tile_pool(name="ln_sts", bufs=1) as stss, \
                 tc.tile_pool(name="ln_bc", bufs=1, space="PSUM") as bcp, \
                 tc.tile_pool(name="ln_tmp", bufs=2) as tmpp, \
                 tc.tile_pool(name="wv_sb", bufs=1) as wvp, \
                 tc.tile_pool(name="v_ps", bufs=1, space="PSUM") as vps:
                wv_sbs = [wvp.tile([128, NCT, 2, 256], bf16, tag=f"wv{g}",
                                   name=f"wv_sb{g}")
                          for g in range(2)]

                def wv_fetch():
                    # deferred so the x load keeps full DMA bandwidth first
                    for g in range(2):
                        for q in range(2):
                            nc.sync.dma_start(
                                out=wv_sbs[g][:, :, q, :],
                                in_=wv[2 * g + q]
                                    .rearrange("(ct p) d -> p ct d", p=128))

                def v_block(tts):
                    # ct-inner so the first tile's chains consume LN apply
                    # output per-ct (no wait for the full normalize loop)
                    for tt in tts:
                        pv0 = vps.tile([128, 512], f32, tag="pv0")
                        pv1 = vps.tile([128, 512], f32, tag="pv1")
                        for ct in range(NCT):
                            nc.tensor.matmul(
                                pv0, lhsT=hT[:, ct, tt * 128:(tt + 1) * 128],
                                rhs=wv_sbs[0][:, ct, :, :],
                                start=(ct == 0), stop=(ct == NCT - 1))
                            nc.tensor.matmul(
                                pv1, lhsT=hT[:, ct, tt * 128:(tt + 1) * 128],
                                rhs=wv_sbs[1][:, ct, :, :],
                                start=(ct == 0), stop=(ct == NCT - 1))
                        for g, pv in ((0, pv0), (1, pv1)):
                            nc.scalar.copy(
                                out=v_all[:, tt, 8 * g:8 * (g + 1), 0:64],
                                in_=pv[:, :].rearrange("p (h d) -> p h d", d=64))

                for th, vt in ((0, range(0, 4)), (1, range(4, 8))):
                    tsl = slice(th * 512, (th + 1) * 512)
                    stf = ln_stats_full(lambda ct, s=tsl: xT[:, ct, s], 512,
                                        sqp, stps, f"h{th}")
                    if th == 0:
                        wv_fetch()
                    keep_warm(wmp, 25)   # statmath runs on DVE/ACT
                    sm = ln_statmath(stf, 512, stss)
                    if th == 1:
                        act_preload(AF.Exp)  # load exp set during V blocks
                    mu_bc, rs_bc = ln_bc(sm, 512, bcp, f"h{th}")
                    keep_warm(wmp, 25)   # normalize ramp on DVE
                    ln_apply(lambda ct, s=tsl: xT[:, ct, s],
                             lambda ct, s=tsl: hT[:, ct, s], 512,
                             mu_bc, rs_bc, g1s, be1s, sz.get('ln1', False), tmpp)
                    v_block(vt)
                    # contiguous copy of own-token columns: Q-projection rhs
                    # streams at full rate instead of through a strided AP
                    for ct in range(NCT):
                        nc.scalar.copy(
                            out=ownhT[:, ct, 256 * th:256 * (th + 1)],
                            in_=hT[:, ct, th * 512:(th + 1) * 512].rearrange(
                                "p (j two i) -> p j two i", two=2, i=64)[:, :, 0, :])
            nc.vector.memset(v_all[:, :, :, 64:65], 1.0)

            # ---------- attention (kt-major) ----------
            with tc.tile_pool(name="wqk", bufs=2) as wqkp, \
                 tc.tile_pool(name="qk_ps", bufs=1, space="PSUM") as qkps, \
                 tc.tile_pool(name="qk_sb", bufs=2) as qksb, \
                 tc.tile_pool(name="s_ps", bufs=2, space="PSUM") as scps, \
                 tc.tile_pool(name="p_sb", bufs=2) as pallp, \
                 tc.tile_pool(name="r_tmp", bufs=2) as rtp, \
                 tc.tile_pool(name="ctx_ps", bufs=2, space="PSUM") as cps:
                def make_qk_parts(hp):
                    wq_sb = wqkp.tile([128, NCT, 128], bf16, tag="wq")
                    nc.sync.dma_start(
                        out=wq_sb, in_=wq[hp].rearrange("(ct p) d -> p ct d", p=128))
                    wk_sb = wqkp.tile([128, NCT, 128], bf16, tag="wk")
                    nc.sync.dma_start(
                        out=wk_sb, in_=wk[hp].rearrange("(ct p) d -> p ct d", p=128))
                    qT = qksb.tile([128, TOK], bf16, tag="qT")
                    kT = qksb.tile([128, T], bf16, tag="kT")

                    def part_q():
                        pq = qkps.tile([128, TOK], f32, tag="pq")
                        for ct in range(NCT):
                            nc.tensor.matmul(pq, lhsT=wq_sb[:, ct, :],
                                             rhs=ownhT[:, ct, :],
                                             start=(ct == 0), stop=(ct == NCT - 1))
                        nc.vector.tensor_copy(out=qT, in_=pq)

                    def part_k(hh):
                        sl = slice(hh * 512, (hh + 1) * 512)
                        pk = qkps.tile([128, TOK], f32, tag="pk")
                        for ct in range(NCT):
                            nc.tensor.matmul(pk, lhsT=wk_sb[:, ct, :],
                                             rhs=hT[:, ct, sl],
                                             start=(ct == 0), stop=(ct == NCT - 1))
                        nc.vector.tensor_copy(out=kT[:, sl], in_=pk)

                    return qT, kT, (part_q, lambda: part_k(0), lambda: part_k(1))

                nc.sync.dma_start(out=sel100, in_=sel100p[:, :, :])
                qT, kT, parts = make_qk_parts(0)
                for pf in parts:
                    pf()

                def r_quarter(q):
                    lo, hi = 32 * q, 32 * q + 4
                    nc.vector.reciprocal(r_all[lo:hi, :], d_all[lo:hi, :])
                    nc.vector.tensor_copy(out=r_hi[lo:hi, :], in_=r_all[lo:hi, :])

                def pmap(g):
                    return 32 * (g // 4) + g % 4

                for hp in range(NP):
                    nxt = make_qk_parts(hp + 1) if hp + 1 < NP else None
                    filler = list(nxt[2]) if nxt else []
                    P = pallp.tile([128, NTT, 2, TOK], bf16, tag="P")
                    pctx0 = cps.tile([65, TOK], f32, tag="pctx", name="pctx0")
                    pctx1 = cps.tile([65, TOK], f32, tag="pctx", name="pctx1")
                    pctxs = (pctx0, pctx1)
                    for kt in range(NTT):
                        off = qoff(kt)
                        sc = scps.tile([128, 2, TOK], f32, tag="sc")
                        nc.tensor.matmul(
                            sc[:, 0, off:],
                            lhsT=kT[0:64, kt * 128:(kt + 1) * 128],
                            rhs=qT[0:64, off:], start=True, stop=True)
                        nc.tensor.matmul(
                            sc[:, 1, off:],
                            lhsT=kT[64:128, kt * 128:(kt + 1) * 128],
                            rhs=qT[64:128, off:], start=True, stop=True)
                        nc.scalar.activation(P[:, kt, :, off:], sc[:, :, off:],
                                             AF.Exp, scale=SM_SCALE)
                        if causal:
                            if kt % 2 == 1:
                                m = kt // 2
                                nc.vector.tensor_mul(
                                    P[:, 2 * m:2 * m + 2, :, 128 * m:128 * (m + 1)],
                                    P[:, 2 * m:2 * m + 2, :, 128 * m:128 * (m + 1)],
                                    mk)
                        else:
                            nc.vector.tensor_mul(P[:, kt, :, :], P[:, kt, :, :],
                                                 mk[:, kt, :, :])
                        if kt % 2 == 1:
                            # AV for the completed pair rides right behind the
                            # mask — no scores->AV serial drain per head pair
                            m = kt // 2
                            for ktp in (2 * m, 2 * m + 1):
                                offp = qoff(ktp)
                                for h2 in range(2):
                                    nc.tensor.matmul(
                                        pctxs[h2][:, offp:],
                                        lhsT=v_all[:, ktp, 2 * hp + h2, :],
                                        rhs=P[:, ktp, h2, offp:],
                                        start=(ktp == 0), stop=(ktp == NTT - 1),
                                        skip_group_check=True)
                            if filler:
                                filler.pop(0)()
                    while filler:
                        filler.pop(0)()
                    for h2 in range(2):
                        pctx = pctxs[h2]
                        den = rtp.tile([1, TOK], f32, tag="den")
                        nc.vector.tensor_copy(out=den, in_=pctx[64:65, :])
                        g = pmap(2 * hp + h2)
                        nc.sync.dma_start(out=d_all[g:g + 1, :], in_=den)
                        nc.vector.tensor_copy(
                            out=ctxT[64 * h2:64 * (h2 + 1), hp, :],
                            in_=pctx[0:64, :])
                    if hp % 2 == 1:
                        r_quarter(hp // 2)
                    if nxt:
                        qT, kT = nxt[0], nxt[1]

            # ---------- normalize ctx + Wo + residual + LN2 stats ----------
            with tc.tile_pool(name="l2_st", bufs=1, space="PSUM") as stps2, \
                 tc.tile_pool(name="l2_sq", bufs=2) as sqp2, \
                 tc.tile_pool(name="l2_sts", bufs=1) as stss2, \
                 tc.tile_pool(name="l2_tmp", bufs=2) as tmpp2, \
                 contextlib.ExitStack() as wo_ctx:
                wop = wo_ctx.enter_context(tc.tile_pool(name="wo_sb", bufs=2))
                aps = wo_ctx.enter_context(
                    tc.tile_pool(name="a_ps", bufs=1, space="PSUM"))
                nrmps = wo_ctx.enter_context(
                    tc.tile_pool(name="nrm_ps", bufs=1, space="PSUM"))
                act_preload(AF.Sqrt)  # reload sqrt set during normalize/Wo
                keep_warm(nrmps, 40)  # cover quarter-3 reciprocal on DVE
                # normalize ctxT per ci; Wo chains accumulate ci 0-5 first
                # and 6-7 last so the Wo matmul stream starts while quarter 3
                # of the softmax reciprocal is still in flight on the DVE
                def pnrm_mul(ci):
                    qq = ci // 2
                    lo = 32 * qq
                    pnrm = nrmps.tile([128, TOK], f32, tag="pnrm")
                    nc.tensor.matmul(pnrm, lhsT=sel100[lo:lo + 4, ci, :],
                                     rhs=r_hi[lo:lo + 4, :],
                                     tile_position=(lo, 0),
                                     start=True, stop=True)
                    nc.vector.tensor_mul(ctxT[:, ci, :], ctxT[:, ci, :], pnrm)

                for ci in range(6):
                    pnrm_mul(ci)
                st2 = stps2.tile([33, TOK], f32, tag="stat2")
                mean2, ex22 = st2[0:1, :], st2[32:33, :]
                wo_sbs = []
                for cot in range(NCT):
                    wo_sb = wop.tile([128, NCT, 128], bf16, tag=f"wo{cot}",
                                     name=f"wo_sb{cot}")
                    nc.sync.dma_start(
                        out=wo_sb, in_=wo[cot].rearrange("(ct p) d -> p ct d", p=128))
                    wo_sbs.append(wo_sb)
                pas = {}
                for half in range(2):
                    cots = list(range(4 * half, 4 * half + 4))
                    for cot in cots:
                        pas[cot] = aps.tile([128, TOK], f32, tag=f"pa{cot % 4}",
                                            name=f"pa{cot}")
                    for ci in range(6):
                        for cot in cots:
                            nc.tensor.matmul(pas[cot], lhsT=wo_sbs[cot][:, ci, :],
                                             rhs=ctxT[:, ci, :],
                                             start=(ci == 0), stop=False)
                    if half == 0:
                        pnrm_mul(6)
                        pnrm_mul(7)
                    for ci in (6, 7):
                        for cot in cots:
                            nc.tensor.matmul(pas[cot], lhsT=wo_sbs[cot][:, ci, :],
                                             rhs=ctxT[:, ci, :],
                                             start=False, stop=(ci == 7))
                    for cot in cots:
                        pa = pas[cot]
                        if sz.get('bo', False):
                            nc.vector.tensor_add(x2T[:, cot, :], pa, own(xT, cot))
                        else:
                            tmpa = sqp2.tile([128, TOK], f32, tag="tmpa")
                            nc.scalar.activation(tmpa, pa, AF.Identity,
                                                 bias=bos[:, cot:cot + 1], scale=1.0)
                            nc.vector.tensor_add(x2T[:, cot, :], tmpa, own(xT, cot))
                        # LN2 stats ride along to keep the PE stream dense
                        sq2 = sqp2.tile([128, TOK], bf16, tag="sq2")
                        nc.vector.tensor_mul(sq2, x2T[:, cot, :], x2T[:, cot, :])
                        nc.tensor.matmul(mean2, lhsT=ones_colf,
                                         rhs=x2T[:, cot, :],
                                         tile_position=(0, 0),
                                         start=(cot == 0), stop=(cot == NCT - 1))
                        nc.tensor.matmul(ex22, lhsT=ones_col, rhs=sq2,
                                         tile_position=(0, 32),
                                         start=(cot == 0), stop=(cot == NCT - 1))
                wo_ctx.close()  # free Wo-phase PSUM before LN2 broadcast
                with tc.tile_pool(name="l2_bc", bufs=1, space="PSUM") as bcp2, \
                     tc.tile_pool(name="warm2_ps", bufs=1, space="PSUM") as wmp2:
                    keep_warm(wmp2, 45)  # statmath2 + bc2 + normalize ramp
                    sm2 = ln_statmath(st2, TOK, stss2)
                    act_preload(AF.Silu if silu_act else AF.Sigmoid)
                    mu_bc2, rs_bc2 = ln_bc(sm2, TOK, bcp2, "l2")
                    ln_apply(lambda ct: x2T[:, ct, :], lambda ct: h2T[:, ct, :],
                             TOK, mu_bc2, rs_bc2, g2s, be2s,
                             sz.get('ln2', False), tmpp2)

        # ---------- FFN ----------
        with tc.tile_pool(name="g_ps", bufs=2, space="PSUM") as gps, \
             tc.tile_pool(name="vl_ps", bufs=2, space="PSUM") as vlps, \
             tc.tile_pool(name="g_sb", bufs=2) as gsbp, \
             tc.tile_pool(name="gv_sb", bufs=2) as gvp, \
             tc.tile_pool(name="o_ps", bufs=2, space="PSUM") as ops:
            for dc in range(4):
                gv = gvp.tile([128, 8, TOK], bf16, tag="gv")
                for fi in range(8):
                    ft = dc * 8 + fi
                    w1_sb = w12p.tile([128, NCT, 128], bf16, tag="w1")
                    nc.sync.dma_start(
                        out=w1_sb, in_=w1[ft].rearrange("(ct p) d -> p ct d", p=128))
                    w2_sb = w12p.tile([128, NCT, 128], bf16, tag="w2")
                    nc.sync.dma_start(
                        out=w2_sb, in_=w2[ft].rearrange("(ct p) d -> p ct d", p=128))
                    pg = gps.tile([128, TOK], f32, tag="pg")
                    pvl = vlps.tile([128, TOK], f32, tag="pvl")
                    for ct in range(NCT):
                        nc.tensor.matmul(pg, lhsT=w1_sb[:, ct, :],
                                         rhs=h2T[:, ct, :],
                                         start=(ct == 0), stop=(ct == NCT - 1))
                    for ct in range(NCT):
                        nc.tensor.matmul(pvl, lhsT=w2_sb[:, ct, :],
                                         rhs=h2T[:, ct, :],
                                         start=(ct == 0), stop=(ct == NCT - 1))
                    gs_t = gsbp.tile([128, TOK], f32, tag="gs_t")
                    if silu_act:
                        nc.scalar.activation(gs_t, pg, AF.Silu,
                                             bias=b1s[:, ft:ft + 1], scale=1.0)
                    else:
                        nc.scalar.activation(gs_t, pg, AF.Sigmoid,
                                             bias=b1s[:, ft:ft + 1], scale=1.0)
                        if sz.get('b1', False):
                            nc.vector.tensor_mul(gs_t, gs_t, pg)
                        else:
                            xg = gsbp.tile([128, TOK], f32, tag="xg")
                            nc.vector.tensor_scalar_add(xg, pg, b1s[:, ft:ft + 1])
                            nc.vector.tensor_mul(gs_t, gs_t, xg)
                    if sz.get('b2', False):
                        nc.vector.tensor_mul(gv[:, fi, :], pvl, gs_t)
                    else:
                        nc.vector.tensor_scalar_add(gv[:, fi, :], pvl,
                                                    b2s[:, ft:ft + 1])
                        nc.vector.tensor_mul(gv[:, fi, :], gv[:, fi, :], gs_t)
                for cot in range(NCT):
                    w3_sb = w3p.tile([128, 8, 128], bf16, tag="w3")
                    nc.sync.dma_start(
                        out=w3_sb,
                        in_=w3[cot, dc * 1024:(dc + 1) * 1024, :]
                            .rearrange("(ft p) d -> p ft d", p=128))
                    po = ops.tile([128, TOK], f32, tag="po")
                    for fi in range(8):
                        nc.tensor.matmul(po, lhsT=w3_sb[:, fi, :],
                                         rhs=gv[:, fi, :],
                                         start=(fi == 0), stop=(fi == 7))
                    if dc == 0:
                        if sz.get('b3', False):
                            nc.vector.tensor_add(outT[:, cot, :], po, x2T[:, cot, :])
                        else:
                            tmpo = gsbp.tile([128, TOK], f32, tag="tmpo")
                            nc.scalar.activation(tmpo, po, AF.Identity,
                                                 bias=b3s[:, cot:cot + 1], scale=1.0)
                            nc.vector.tensor_add(outT[:, cot, :], tmpo,
                                                 x2T[:, cot, :])
                    else:
                        nc.vector.tensor_add(outT[:, cot, :], outT[:, cot, :], po)
                        if dc == 3:
                            nc.sync.dma_start(
                                out=out[cot * 128:(cot + 1) * 128, :],
                                in_=outT[:, cot, :])
    patch_nc(nc)
    return nc


# ===================== host-side prep =====================

def swap_cols64(a):
    """swap adjacent 64-col blocks along last axis"""
    s = a.shape
    b = a.reshape(*s[:-1], s[-1] // 128, 2, 64)
    return b[..., ::-1, :].reshape(s)


def check_causal(mask):
    T_ = mask.shape[0]
    allow = ~np.isneginf(np.asarray(mask))
    allow_ref = ~np.triu(np.ones((T_, T_), bool), k=1)
    return np.array_equal(allow, allow_ref)


def make_mask_tiles(mask, causal):
    """per-core multiplicative mask tiles (bf16 0/1), key-order swapped for odd
    cores, replicated across the two heads of a pair (h2 axis).

    Causal: [128 k, 2 ktpar, 2 h2, 128 q] — [:,0] masks key tile kt=2m
    ([diag | ones]), [:,1] masks kt=2m+1 ([zeros | diag]); the pattern is
    m-independent. General: [NTT, 128 k, 2 h2, TOK q] per kt over all q."""
    allow = ~np.isneginf(np.asarray(mask))  # [q, k] True = allowed
    tiles = []
    for core in range(8):
        par = core % 2

        def ktile_order(kt):
            k = np.arange(128 * kt, 128 * kt + 128)
            if par == 1:
                k = k.reshape(2, 64)[::-1].reshape(128)
            return k

        def qcols(mp):
            j0, j1 = 2 * (2 * mp) + par, 2 * (2 * mp + 1) + par
            return np.concatenate([np.arange(64 * j0, 64 * j0 + 64),
                                   np.arange(64 * j1, 64 * j1 + 64)])

        if causal:
            mp = 0
            m = np.zeros((128, 2, 2, 128), dtype=ml_dtypes.bfloat16)
            q = qcols(mp)
            for i, kt in enumerate((2 * mp, 2 * mp + 1)):
                blk = allow[np.ix_(q, ktile_order(kt))].T
                m[:, i, 0, :] = blk
                m[:, i, 1, :] = blk
            tiles.append(np.ascontiguousarray(m))
        else:
            qall = np.concatenate([qcols(mp) for mp in range(4)])
            m = np.zeros((NTT, 128, 2, TOK), dtype=ml_dtypes.bfloat16)
            for kt in range(NTT):
                blk = allow[np.ix_(qall, ktile_order(kt))].T
                m[kt, :, 0, :] = blk
                m[kt, :, 1, :] = blk
            tiles.append(m)
    return tiles


def prep_in_maps(inputs):
    bfl = ml_dtypes.bfloat16
    x = np.asarray(inputs['input'], np.float32)      # [B, T, C]
    mask = np.asarray(inputs['mask'], np.float32)
    causal = check_causal(mask)
    Wq = np.asarray(inputs['Wq'], np.float32)        # [H, C, D]
    Wk = np.asarray(inputs['Wk'], np.float32)
    Wv = np.asarray(inputs['Wv'], np.float32)
    Wo = np.asarray(inputs['Wo'], np.float32)        # [C, C]
    w1 = np.asarray(inputs['w1'], np.float32)        # [C, DFF]
    w2 = np.asarray(inputs['w2'], np.float32)
    w3 = np.asarray(inputs['w3'], np.float32)        # [DFF, C]

    wq_l = np.ascontiguousarray(
        Wq.reshape(NP, 2, C, D).transpose(0, 2, 1, 3).reshape(NP, C, 128)).astype(bfl)
    wk_l = np.ascontiguousarray(
        Wk.reshape(NP, 2, C, D).transpose(0, 2, 1, 3).reshape(NP, C, 128)).astype(bfl)
    wv_l = np.ascontiguousarray(
        Wv.reshape(4, 4, C, D).transpose(0, 2, 1, 3).reshape(4, C, 256)).astype(bfl)
    wo_l = np.ascontiguousarray(Wo.reshape(C, NCT, 128).transpose(1, 0, 2)).astype(bfl)
    w1_l = np.ascontiguousarray(w1.reshape(C, 32, 128).transpose(1, 0, 2)).astype(bfl)
    w2_l = np.ascontiguousarray(w2.reshape(C, 32, 128).transpose(1, 0, 2)).astype(bfl)
    w3_l = np.ascontiguousarray(w3.reshape(DFF, NCT, 128).transpose(1, 0, 2)).astype(bfl)

    def packp(v):
        return np.ascontiguousarray(np.asarray(v, np.float32).reshape(-1, 128).T)

    g1p = packp(inputs['g1']); be1p = packp(inputs['be1'])
    g2p = packp(inputs['g2']); be2p = packp(inputs['be2'])
    bop = packp(inputs['bo']); b3p = packp(inputs['b3'])
    b1p = packp(inputs['b1']); b2p = packp(inputs['b2'])

    mask_tiles = make_mask_tiles(mask, causal)

    sel100_h = np.zeros((100, NCT, 128), dtype=ml_dtypes.bfloat16)
    for ci in range(NCT):
        for g, cols in ((2 * ci, slice(0, 64)), (2 * ci + 1, slice(64, 128))):
            sel100_h[32 * (g // 4) + g % 4, ci, cols] = 1

    in_maps = []
    for core in range(8):
        b, par = core // 2, core % 2
        xt_c = np.ascontiguousarray(x[b].T)            # [C, T]
        if par == 1:
            xt_c = np.ascontiguousarray(swap_cols64(xt_c))
        in_maps.append(dict(
            xt=xt_c, maskt=mask_tiles[core],
            wq=wq_l, wk=wk_l, wv=wv_l, wo=wo_l, w1=w1_l, w2=w2_l, w3=w3_l,
            g1p=g1p, be1p=be1p, g2p=g2p, be2p=be2p, bop=bop, b3p=b3p,
            b1p=b1p, b2p=b2p, sel100p=sel100_h))
    szflags = dict(
        ln1=bool(np.all(np.asarray(inputs['g1']) == 1)
                 and np.all(np.asarray(inputs['be1']) == 0)),
        ln2=bool(np.all(np.asarray(inputs['g2']) == 1)
                 and np.all(np.asarray(inputs['be2']) == 0)),
        bo=bool(np.all(np.asarray(inputs['bo']) == 0)),
        b1=bool(np.all(np.asarray(inputs['b1']) == 0)),
        b2=bool(np.all(np.asarray(inputs['b2']) == 0)),
        b3=bool(np.all(np.asarray(inputs['b3']) == 0)),
    )
    return in_maps, causal, szflags


def assemble(outs, B=4):
    """outs: list of 8 per-core dicts with 'out' [C, TOK] -> [B, T, C]"""
    full = np.zeros((B, T, C), np.float32)
    for core in range(8):
        b, par = core // 2, core % 2
        o = np.asarray(outs[core]['out']).reshape(C, NCH, 64)
        for p in range(NCH):
            j = 2 * p + par
            full[b, 64 * j:64 * j + 64, :] = o[:, p, :].T
    return full


# ===================== entry point =====================

_NC_CACHE = {}


def _get_nc(causal, sz):
    key = (causal, tuple(sorted(sz.items())))
    if key not in _NC_CACHE:
        _NC_CACHE[key] = build_nc(causal=causal, sz=sz, silu_act=True)
    return _NC_CACHE[key]


def run_on_hw(inputs):
    from concourse import bass2jax
    in_maps, causal, sz = prep_in_maps(inputs)
    nc = _get_nc(causal, sz)
    results = bass2jax.run_bass_via_pjrt(nc, in_maps, n_cores=8)
    return assemble(results)


def kernel(**inputs):
    return run_on_hw(inputs)
